# revision 1
# baseline (speedup 1.0000x reference)
"""Multi-head attention (B=2, S=2048, d_model=768, 12 heads) on 8 trn2 cores.

Sharding: 24 (batch, head) pairs -> 3 heads + 1 batch per core.
Per-core device pipeline (fp32 data, fp32r matmuls):
  - host supplies x.T (feature-major) per batch, and per-head weight slices
    packed in matmul-ready lhsT layouts (h2 duplicated into both partition
    halves so scores can run two concurrent 64-contraction matmuls via PE
    row tiling)
  - projections produce Q^T, K^T, V^T [dk, q]
  - V^T is PE-transposed back to V [k, dk]; the key mask is applied
    multiplicatively to V rows (exp(s + mask*-1e9) == exp(s) * m01[k]), and
    a masked ones-column appended to V computes the softmax denominator in
    the same P@V matmul (output row 64)
  - scores S^T[k, q] per 128-k block; exp on ScalarE with the 1/sqrt(dk)
    scale folded in; P@V accumulates O'^T and the row-sum in PSUM
  - the row-sum reciprocal is broadcast across 64 partitions with a K=1
    matmul; O^T = O'^T * recip; the output projection consumes O^T directly
  - host sums the 4 per-core partials of each batch and adds b_o.

The TPB instruction encoding holds a single sync-wait slot; this walrus
build refuses instructions whose BIR sync_info carries more than one wait.
_legalize_sync() splits extra waits into single-wait NoOps placed just
before the instruction on the same engine queue (queues are in-order, so
semantics are identical).
"""

import json
import sys

for _p in ("/opt/trn_rl_repo",):
    if _p not in sys.path:
        sys.path.insert(0, _p)

import numpy as np

import concourse.bass as bass
import concourse.mybir as mybir
from concourse.tile import TileContext
from concourse.bass_utils import run_bass_kernel_spmd

D_MODEL = 768
N_HEADS = 12
DK = 64
B = 2
SQ = 2048
SK = 2048
HPC = 3  # heads per core
N_CORES = 8
FC = D_MODEL // 128  # 6 f-chunks of 128
QT_TILES = SQ // 128  # 16
KB = SK // 128  # 16 key blocks
QC = SQ // 512  # 4 query chunks of 512

F32 = mybir.dt.float32
F32R = mybir.dt.float32r


def _legalize_sync(bj):
    """Split >1-wait instructions into single-wait NoOps + the instruction."""
    n = 0
    for fn in bj["functions"]:
        for blk in fn["blocks"]:
            out = []
            for inst in blk["instructions"]:
                si = inst.get("sync_info") or None
                waits = (si or {}).get("on_wait") or []
                if len(waits) > 1:
                    # merge duplicate semaphores (keep max wait_value)
                    merged = {}
                    for w in waits:
                        k = w.get("id", w.get("ant_name"))
                        if k not in merged or w.get("wait_value", 0) > merged[
                            k
                        ].get("wait_value", 0):
                            merged[k] = w
                    waits = list(merged.values())
                if len(waits) > 1:
                    for w in waits[:-1]:
                        n += 1
                        out.append(
                            {
                                "engine": inst["engine"],
                                "ins": [],
                                "name": f"I-syncfix-{n}",
                                "opcode": "NoOp",
                                "outs": [],
                                "sync_info": {"on_update": [], "on_wait": [w]},
                            }
                        )
                    si["on_wait"] = [waits[-1]]
                out.append(inst)
            blk["instructions"] = out
    return bj


class _Bass(bass.Bass):
    def to_json_bytes(self):
        bj = json.loads(super().to_json_bytes())
        return json.dumps(_legalize_sync(bj)).encode()


def build_nc(stage=4, qc_n=QC, kb_n=KB, qt_n=QT_TILES, att_reps=1):
    nc = _Bass()

    xtq = nc.dram_tensor("xtq", [D_MODEL, SQ], F32R, kind="ExternalInput")
    xtk = nc.dram_tensor("xtk", [D_MODEL, SK], F32R, kind="ExternalInput")
    xtv = nc.dram_tensor("xtv", [D_MODEL, SK], F32R, kind="ExternalInput")
    wq = nc.dram_tensor("wq", [128, FC * 2 * 128], F32R, kind="ExternalInput")
    wk = nc.dram_tensor("wk", [128, FC * 2 * 128], F32R, kind="ExternalInput")
    wv = nc.dram_tensor("wv", [128, FC * 2 * 128], F32R, kind="ExternalInput")
    wo = nc.dram_tensor("wo", [128, 2 * D_MODEL], F32R, kind="ExternalInput")
    bq = nc.dram_tensor("bq", [128, 2], F32, kind="ExternalInput")
    bk = nc.dram_tensor("bk", [128, 2], F32, kind="ExternalInput")
    bv = nc.dram_tensor("bv", [128, 2], F32, kind="ExternalInput")
    m01 = nc.dram_tensor("m01", [SK], F32, kind="ExternalInput")
    idin = nc.dram_tensor("idin", [128, 128], F32R, kind="ExternalInput")
    onesin = nc.dram_tensor("onesin", [1, 64], F32R, kind="ExternalInput")
    out = nc.dram_tensor("out", [SQ, D_MODEL], F32, kind="ExternalOutput")

    with TileContext(nc) as tc, nc.allow_low_precision(reason="fp32r pipeline"):
        with (
            tc.tile_pool(name="singles", bufs=1) as singles,
            tc.tile_pool(name="xts", bufs=3) as xts,
            tc.tile_pool(name="exps", bufs=5) as exps,
            tc.tile_pool(name="rcp", bufs=2) as rcps,
            tc.tile_pool(name="outs", bufs=4) as outs,
        ):
            # ---- load constants / weights -------------------------------
            wq_sb = singles.tile([128, FC, 2, 128], F32R)
            wk_sb = singles.tile([128, FC, 2, 128], F32R)
            wv_sb = singles.tile([128, FC, 2, 128], F32R)
            wo_sb = singles.tile([128, 2 * D_MODEL], F32R)
            bq_sb = singles.tile([128, 2], F32)
            bk_sb = singles.tile([128, 2], F32)
            bv_sb = singles.tile([128, 2], F32)
            m01_sb = singles.tile([128, KB], F32)
            ident = singles.tile([128, 128], F32R)
            ones_sb = singles.tile([1, 64], F32R)

            nc.sync.dma_start(
                out=wq_sb, in_=wq.rearrange("p (a b c) -> p a b c", a=FC, b=2)
            )
            nc.sync.dma_start(
                out=wk_sb, in_=wk.rearrange("p (a b c) -> p a b c", a=FC, b=2)
            )
            nc.sync.dma_start(
                out=wv_sb, in_=wv.rearrange("p (a b c) -> p a b c", a=FC, b=2)
            )
            nc.sync.dma_start(out=wo_sb, in_=wo[:])
            nc.sync.dma_start(out=bq_sb, in_=bq[:])
            nc.sync.dma_start(out=bk_sb, in_=bk[:])
            nc.sync.dma_start(out=bv_sb, in_=bv[:])
            nc.sync.dma_start(out=m01_sb, in_=m01.rearrange("(t p) -> p t", p=128))
            nc.sync.dma_start(out=ident, in_=idin[:])
            nc.sync.dma_start(out=ones_sb, in_=onesin[:])

            # persistent activations
            qt_sb = singles.tile([128, 2, SQ], F32R)  # Q^T (ch0: h0|h1, ch1: h2|h2)
            kt_sb = singles.tile([128, 2, SK], F32R)  # K^T
            vt_sb = singles.tile([128, 2, SK], F32R)  # V^T (ch1 rows 64.. junk)
            vaug_sb = singles.tile([128, HPC, KB, 65], F32R)  # masked V + mask col
            ot_sb = singles.tile([128, 2, SQ], F32R)  # normalized O^T

            # ---- projections -------------------------------------------
            def project(xt_dram, w_sb, b_sb, dst_sb, ch1_m):
                with tc.tile_pool(name="pp_proj", bufs=8, space="PSUM") as pp:
                    ps = {}
                    for ch in range(2):
                        for qc in range(QC):
                            ps[(ch, qc)] = pp.tile(
                                [128, 512], F32, tag="proj_ps", name=f"pps{ch}{qc}"
                            )
                    for fc in range(FC):
                        xchunk = xts.tile([128, SQ], F32R, tag="xchunk")
                        nc.sync.dma_start(
                            out=xchunk, in_=xt_dram[fc * 128 : (fc + 1) * 128, :]
                        )
                        for ch in range(2):
                            m = 128 if ch == 0 else ch1_m
                            for qc in range(QC):
                                nc.tensor.matmul(
                                    ps[(ch, qc)][:m, :],
                                    w_sb[:, fc, ch, :m],
                                    xchunk[:, qc * 512 : (qc + 1) * 512],
                                    start=(fc == 0),
                                    stop=(fc == FC - 1),
                                )
                    for ch in range(2):
                        m = 128 if ch == 0 else ch1_m
                        for qc in range(QC):
                            nc.vector.tensor_scalar_add(
                                dst_sb[:m, ch, qc * 512 : (qc + 1) * 512],
                                ps[(ch, qc)][:m, :],
                                b_sb[:m, ch : ch + 1],
                            )

            # V first so attention is unblocked early; then Q, K.
            project(xtv, wv_sb, bv_sb, vt_sb, 64)
            project(xtq, wq_sb, bq_sb, qt_sb, 128)
            project(xtk, wk_sb, bk_sb, kt_sb, 128)

            # ---- V^T -> V_aug (transpose + mask + ones col) -------------
            # One PSUM bank per transpose: row-tiled transposes execute
            # concurrently in the PE array, and concurrent PE writes into a
            # shared bank are fatal on hardware.
            with tc.tile_pool(name="pp_vt", bufs=6, space="PSUM") as ppv:
                for kt in range(KB if stage >= 2 else 0):
                    sl = slice(kt * 128, (kt + 1) * 128)
                    vs = []
                    for h in range(HPC):
                        t = ppv.tile([128, 64], F32R, tag="vstage", name=f"vs{h}")
                        vs.append(t)
                    # h0: VT[0:64, ch0], h1: VT[64:128, ch0], h2: VT[0:64, ch1]
                    nc.tensor.transpose(
                        vs[0], vt_sb[0:64, 0, sl], ident[0:64, 0:64]
                    )
                    nc.tensor.transpose(
                        vs[1],
                        vt_sb[64:128, 0, sl],
                        ident[64:128, 64:128],
                        tile_position=(64, 0),
                    )
                    nc.tensor.transpose(
                        vs[2], vt_sb[0:64, 1, sl], ident[0:64, 0:64]
                    )
                    for h in range(HPC):
                        nc.vector.tensor_scalar_mul(
                            vaug_sb[:, h, kt, 0:64],
                            vs[h],
                            m01_sb[:, kt : kt + 1],
                        )
                    # mask column (broadcast m01 over the 3 heads)
                    mcol = m01_sb[:, kt : kt + 1]
                    bcast = bass.AP(
                        tensor=mcol.tensor,
                        offset=mcol.offset,
                        ap=[mcol.ap[0], [0, HPC], [0, 1]],
                    )
                    nc.vector.tensor_copy(vaug_sb[:, :, kt, 64:65], bcast)

            # ---- attention ---------------------------------------------
            with (
                tc.tile_pool(name="pp_st", bufs=2, space="PSUM") as pst,
                tc.tile_pool(name="pp_o", bufs=3, space="PSUM") as po,
                tc.tile_pool(name="pp_rs", bufs=1, space="PSUM") as prs,
            ):

                def normalize(h, qc, o_ps):
                    """OT[...] = O'[0:64] * (1/rs) ; rs = O'[64]"""
                    rs_rcp = rcps.tile([1, 512], F32R, tag="rs_rcp")
                    nc.vector.reciprocal(rs_rcp, o_ps[64:65, :])
                    rsmat = prs.tile([128, 512], F32, tag="rsmat")
                    nc.tensor.matmul(
                        rsmat[0:64, :], ones_sb, rs_rcp, start=True, stop=True
                    )
                    rcpm = rcps.tile([64, 512], F32, tag="rcpm")
                    nc.vector.tensor_copy(rcpm, rsmat[0:64, :])
                    ch, r0 = ((0, 0), (0, 64), (1, 0))[h]
                    nc.vector.tensor_mul(
                        ot_sb[r0 : r0 + 64, ch, qc * 512 : (qc + 1) * 512],
                        o_ps[0:64, :],
                        rcpm,
                    )

                def h01_step(qsl, qc, o0, o1, kb):
                    ksl = slice(kb * 128, (kb + 1) * 128)
                    stp = pst.tile([128, 1024], F32, tag="stp", name="stp")
                    nc.tensor.matmul(
                        stp[:, 0:512],
                        kt_sb[0:64, 0, ksl],
                        qt_sb[0:64, 0, qsl],
                        start=True,
                        stop=True,
                    )
                    nc.tensor.matmul(
                        stp[:, 512:1024],
                        kt_sb[64:128, 0, ksl],
                        qt_sb[64:128, 0, qsl],
                        start=True,
                        stop=True,
                        tile_position=(64, 0),
                    )
                    est = exps.tile([128, 1024], F32R, tag="est", name="est")
                    nc.scalar.activation(
                        est, stp, mybir.ActivationFunctionType.Exp, scale=0.125
                    )
                    nc.tensor.matmul(
                        o0[0:65, :],
                        vaug_sb[:, 0, kb, :],
                        est[:, 0:512],
                        start=(kb == 0),
                        stop=(kb == KB - 1),
                    )
                    nc.tensor.matmul(
                        o1[0:65, :],
                        vaug_sb[:, 1, kb, :],
                        est[:, 512:1024],
                        start=(kb == 0),
                        stop=(kb == KB - 1),
                    )

                def h2_step(qsl, qc, o2, kp):
                    ka = slice((2 * kp) * 128, (2 * kp + 1) * 128)
                    kb_ = slice((2 * kp + 1) * 128, (2 * kp + 2) * 128)
                    stp = pst.tile([128, 1024], F32, tag="stp", name="stp")
                    nc.tensor.matmul(
                        stp[:, 0:512],
                        kt_sb[0:64, 1, ka],
                        qt_sb[0:64, 1, qsl],
                        start=True,
                        stop=True,
                    )
                    nc.tensor.matmul(
                        stp[:, 512:1024],
                        kt_sb[64:128, 1, kb_],
                        qt_sb[64:128, 1, qsl],
                        start=True,
                        stop=True,
                        tile_position=(64, 0),
                    )
                    est = exps.tile([128, 1024], F32R, tag="est", name="est")
                    nc.scalar.activation(
                        est, stp, mybir.ActivationFunctionType.Exp, scale=0.125
                    )
                    nc.tensor.matmul(
                        o2[0:65, :],
                        vaug_sb[:, 2, 2 * kp, :],
                        est[:, 0:512],
                        start=(kp == 0),
                        stop=False,
                    )
                    nc.tensor.matmul(
                        o2[0:65, :],
                        vaug_sb[:, 2, 2 * kp + 1, :],
                        est[:, 512:1024],
                        start=False,
                        stop=(kp == KB // 2 - 1),
                    )

                for _rep in range(att_reps):
                  for qc in range(qc_n if stage >= 3 else 0):
                    qsl = slice(qc * 512, (qc + 1) * 512)
                    # all three heads interleaved: h0/h1 every kb, one h2
                    # pair-step every other kb — keeps ScalarE fed with no
                    # section-boundary drain
                    o0 = po.tile([128, 512], F32, tag="o_ps", name="o0")
                    o1 = po.tile([128, 512], F32, tag="o_ps", name="o1")
                    o2 = po.tile([128, 512], F32, tag="o_ps", name="o2")
                    for kb in range(KB):
                        h01_step(qsl, qc, o0, o1, kb)
                        if kb % 2 == 1:
                            h2_step(qsl, qc, o2, kb // 2)
                    normalize(0, qc, o0)
                    normalize(1, qc, o1)
                    normalize(2, qc, o2)

            # ---- output projection -------------------------------------
            with (
                tc.tile_pool(name="pp_out1", bufs=2, space="PSUM") as pout1,
                tc.tile_pool(name="pp_out2", bufs=2, space="PSUM") as pout2,
            ):
                for qt in range(qt_n if stage >= 4 else 0):
                    qsl = slice(qt * 128, (qt + 1) * 128)
                    ps1 = pout1.tile([128, 512], F32, tag="ops1")
                    ps2 = pout2.tile([128, 256], F32, tag="ops2")
                    nc.tensor.matmul(
                        ps1, ot_sb[:, 0, qsl], wo_sb[:, 0:512],
                        start=True, stop=False,
                    )
                    nc.tensor.matmul(
                        ps1, ot_sb[0:64, 1, qsl], wo_sb[0:64, 768:1280],
                        start=False, stop=True,
                    )
                    nc.tensor.matmul(
                        ps2, ot_sb[:, 0, qsl], wo_sb[:, 512:768],
                        start=True, stop=False,
                    )
                    nc.tensor.matmul(
                        ps2, ot_sb[0:64, 1, qsl], wo_sb[0:64, 1280:1536],
                        start=False, stop=True,
                    )
                    osb = outs.tile([128, D_MODEL], F32, tag="osb")
                    nc.vector.tensor_copy(osb[:, 0:512], ps1)
                    nc.vector.tensor_copy(osb[:, 512:768], ps2)
                    nc.sync.dma_start(out=out[qsl, :], in_=osb)

    return nc


# ---------------- host-side prep / gather ----------------------------------


def _prep_w(w, hd, dup):
    """lhsT layout [128 f, FC, 2, 128 m] for W rows hd (192 head dims)."""
    wh = np.asarray(w, np.float32)[hd, :]  # [192, 768]
    s1 = wh[0:128]
    if dup:
        s2 = np.concatenate([wh[128:192], wh[128:192]], axis=0)
    else:
        s2 = np.concatenate([wh[128:192], np.zeros((64, D_MODEL), np.float32)], axis=0)
    arr = np.stack([s1, s2], axis=0)  # [2, 128m, 768f]
    arr = arr.reshape(2, 128, FC, 128)  # [ch, m, fc, f]
    arr = np.ascontiguousarray(arr.transpose(3, 2, 0, 1))  # [f, fc, ch, m]
    return arr.reshape(128, FC * 2 * 128)


def _prep_b(b, hd, dup):
    bh = np.asarray(b, np.float32)[hd]
    c0 = bh[0:128]
    if dup:
        c1 = np.concatenate([bh[128:192], bh[128:192]])
    else:
        c1 = np.concatenate([bh[128:192], np.zeros(64, np.float32)])
    return np.ascontiguousarray(np.stack([c0, c1], axis=1))  # [128, 2]


def make_in_maps(q, k, v, mask, w_q, b_q, w_k, b_k, w_v, b_v, w_o):
    q = np.asarray(q, np.float32)
    k = np.asarray(k, np.float32)
    v = np.asarray(v, np.float32)
    mask = np.asarray(mask)
    in_maps = []
    for c in range(N_CORES):
        b = c // 4
        h0 = (c % 4) * HPC
        hd = np.arange(h0 * DK, (h0 + HPC) * DK)
        woc = np.asarray(w_o, np.float32)[:, hd]  # [768, 192]
        wot = np.ascontiguousarray(woc.T)  # [192, 768]
        wo_prep = np.zeros((128, 2 * D_MODEL), np.float32)
        wo_prep[:, 0:D_MODEL] = wot[0:128]
        wo_prep[0:64, D_MODEL:] = wot[128:192]
        in_maps.append(
            {
                "xtq": np.ascontiguousarray(q[b].T),
                "xtk": np.ascontiguousarray(k[b].T),
                "xtv": np.ascontiguousarray(v[b].T),
                "wq": _prep_w(w_q, hd, True),
                "wk": _prep_w(w_k, hd, True),
                "wv": _prep_w(w_v, hd, False),
                "wo": wo_prep,
                "bq": _prep_b(b_q, hd, True),
                "bk": _prep_b(b_k, hd, True),
                "bv": _prep_b(b_v, hd, False),
                "m01": (mask[b] != 0).astype(np.float32),
                "idin": np.eye(128, dtype=np.float32),
                "onesin": np.ones((1, 64), np.float32),
            }
        )
    return in_maps


_NC_CACHE = {}


def kernel(q, k, v, mask, w_q, b_q, w_k, b_k, w_v, b_v, w_o, b_o, **kw):
    in_maps = make_in_maps(q, k, v, mask, w_q, b_q, w_k, b_k, w_v, b_v, w_o)
    if "nc" not in _NC_CACHE:
        _NC_CACHE["nc"] = build_nc()
    nc = _NC_CACHE["nc"]
    res = run_bass_kernel_spmd(nc, in_maps, core_ids=list(range(N_CORES)))
    parts = [r["out"] for r in res.results]
    b_o = np.asarray(b_o, np.float32)
    full = np.empty((B, SQ, D_MODEL), np.float32)
    for b in range(B):
        acc = parts[4 * b].astype(np.float32).copy()
        for c in range(4 * b + 1, 4 * b + 4):
            acc += parts[c]
        full[b] = acc + b_o[None, :]
    return full


def build_calib_nc():
    """Same external inputs as build_nc, near-zero compute: for subtracting
    transfer/dispatch overhead from wall-clock timing."""
    nc = _Bass()
    names = [
        ("xtq", [D_MODEL, SQ], F32R), ("xtk", [D_MODEL, SK], F32R),
        ("xtv", [D_MODEL, SK], F32R), ("wq", [128, FC * 2 * 128], F32R),
        ("wk", [128, FC * 2 * 128], F32R), ("wv", [128, FC * 2 * 128], F32R),
        ("wo", [128, 2 * D_MODEL], F32R), ("bq", [128, 2], F32),
        ("bk", [128, 2], F32), ("bv", [128, 2], F32), ("m01", [SK], F32),
        ("idin", [128, 128], F32R), ("onesin", [1, 64], F32R),
    ]
    handles = {n: nc.dram_tensor(n, s, d, kind="ExternalInput") for n, s, d in names}
    out = nc.dram_tensor("out", [SQ, D_MODEL], F32, kind="ExternalOutput")
    with TileContext(nc) as tc:
        with tc.tile_pool(name="s", bufs=1) as s:
            t = s.tile([128, 128], F32R)
            nc.sync.dma_start(out=t, in_=handles["idin"][:])
            nc.sync.dma_start(out=out[0:128, 0:128], in_=t.bitcast(F32))
    return nc

